# revision 50
# baseline (speedup 1.0000x reference)
"""BertLayer (attention + adapter + FFN + LayerNorm) Trainium2 Bass kernel.

Sharding: 8 cores, pure SPMD (no collectives). Core c handles batch b=c//4
and query rows [q0, q0+512) with q0=(c%4)*512. Each core computes K/V for
its full batch locally (replicated within the 4-core batch group), then
attention / adapter / FFN / LayerNorm for its 512 rows.

All on-chip compute is done in the "transposed" orientation (feature dim
on partitions, token dim on the free axis) so that every matmul has its
contraction dim on partitions and no on-device transposes are needed.

Structure (v3):
- The attention mask is folded into V: V rows (and the denominator ones
  column appended to V) are scaled by exp(mask[kpos]), which is exact
  (softmax(s+m) = diag(e^m)exp(s)/sum(e^m exp(s))). The softmax exp then
  has a CONSTANT bias (-2 fp16-overflow guard, cancels in normalization),
  letting one ScalarE activation cover a [128, 4*512] PSUM tile (a head
  pair x 2 kpos tiles) - few, wide ACT ops on the critical engine.
- Attention is chunk-major: for each 512-kpos chunk, all 12 heads'
  scores+exp+context run, with per-head context (and softmax denominator)
  accumulated across chunks into SBUF fp16 tiles by VectorE. K/V
  projection chains for chunk c+1 are interleaved between the head pairs
  of chunk c, so the PE always has independent work while ScalarE runs
  the exp chain.
- Scores matmuls for head pairs (2h, 2h+1) use disjoint 64-row groups of
  the PE array (base partitions 0/64) and separate PSUM banks, so the
  hardware runs them concurrently.
- All matmul operands are fp16 with fp32 PSUM accumulation. (fp8 was
  tried and rejected: the pre-LayerNorm signal is small, so LayerNorm
  renormalizes and fp8's ~5% per-projection error passes through to the
  output at full relative strength - measured 7e-2 vs the 2e-2 gate.)
- LayerNorm is folded into two small matmuls: A = g (x) rs and
  M = g (x) nm + b (x) 1 (rank-1/2 broadcasts), leaving 2 VectorE
  tensor-tensor ops per output block. FFN2 is m-outer (all gelu(it_k)
  tiles kept resident) so the residual/square/mean-accumulation work
  pipelines during FFN2 instead of forming a serial tail.
"""

import numpy as np

import concourse.bass as bass
import concourse.mybir as mybir
import concourse.tile as tile
from concourse import bacc
from concourse.bass_utils import run_bass_kernel_spmd
from contextlib import ExitStack

F32 = mybir.dt.float32
F16 = mybir.dt.float16
AF = mybir.ActivationFunctionType
ALU = mybir.AluOpType

B, S, H = 2, 2048, 768
NH, DH = 12, 64
FF = 3072
AD = 64
EPS = 1e-12
P = 128
KO = H // P          # 6 partition-tiles of the hidden dim
Q = 512              # query rows per core
NCORES = 8
NCH = 4              # kpos chunks (512 each)
CH = S // NCH        # 512
JT = CH // P         # 4 kpos 128-tiles per chunk
FFT = FF // P        # 24
VH = 65              # per-head V columns incl. exp(mask) column
GEXP = -2.0          # exp overflow guard; cancels in normalization


def _build_nc():
    nc = bacc.Bacc(
        "TRN2",
        target_bir_lowering=False,
        debug=False,
        num_devices=NCORES,
    )

    def din(name, shape, dt=F32):
        return nc.dram_tensor(name, list(shape), dt, kind="ExternalInput").ap()

    xt = din("xt", (H, S), F16)         # hidden[b].T
    xtq = din("xtq", (H, Q), F16)       # hidden[b, q0:q0+Q].T
    wqt = din("wqt", (H, H), F16)
    wkt = din("wkt", (H, H), F16)
    wvt = din("wvt", (H, H), F16)
    afit = din("afit", (H, AD), F16)
    aset = din("aset", (AD, H), F16)
    wit = din("wit", (H, FF), F16)
    wot = din("wot", (FF, H), F16)
    consts = din("consts", (P, 83))
    lnr = din("lnr", (2, H), F16)      # row0 = ln_g, row1 = ln_b
    outt = nc.dram_tensor("outt", [H, Q], F16, kind="ExternalOutput").ap()

    def part6(ap):  # [(ko p), n] -> [p, ko, n]
        return ap.rearrange("(ko p) n -> p ko n", p=P)

    with tile.TileContext(nc) as tc, nc.allow_low_precision(
        reason="fp8/fp16 matmul operands; accumulation stays fp32 in PSUM"
    ), ExitStack() as top:
        const = top.enter_context(tc.tile_pool(name="const", bufs=1))
        persist = top.enter_context(tc.tile_pool(name="persist", bufs=1))

        consts_sb = const.tile([P, 83], F32, tag="consts")
        em_sb = consts_sb[:, 0:16]         # exp(mask) per kpos tile
        bq_sb = consts_sb[:, 16:22]
        bk_sb = consts_sb[:, 22:28]
        bv_sb = consts_sb[:, 28:34]
        aseb_sb = consts_sb[:, 34:40]
        bo_sb = consts_sb[:, 40:46]
        bi_sb = consts_sb[:, 58:82]
        afib_sb = consts_sb[0:AD, 82:83]
        ones_col = const.tile([P, 1], F16, tag="ones")
        nc.vector.memset(ones_col[:], 1.0)
        ones_row = const.tile([1, P], F16, tag="ones_row")
        nc.vector.memset(ones_row[:], 1.0)
        gexp_col = const.tile([P, 1], F32, tag="gexp")
        nc.vector.memset(gexp_col[:], GEXP)
        eps_sb = const.tile([1, 1], F32, tag="eps")
        nc.vector.memset(eps_sb[:], EPS)
        lnr_sb = const.tile([2, H], F16, tag="lnr")

        # normalized attention output, transposed [H, Q]
        aot = [
            persist.tile([P, Q], F16, tag=f"aot{m}", name=f"aot{m}")
            for m in range(KO)
        ]
        aw_pool = top.enter_context(tc.tile_pool(name="aw", bufs=1))
        wi_pool = top.enter_context(tc.tile_pool(name="wi", bufs=1))

        # ========== stage 0 + 1: QKV projections + attention ==========
        with ExitStack() as s01:
            xt_pool = s01.enter_context(tc.tile_pool(name="xt", bufs=1))
            qt_pool = s01.enter_context(tc.tile_pool(name="qt", bufs=1))
            QT = qt_pool.tile([P, KO, Q], F16, tag="QT")

            xtp = part6(xt)
            xtc = {}

            # --- DMAs, ordered so K-projection (first PE work) unblocks
            # earliest: wk + the first xt chunk, then Q/V inputs
            w0_pool = s01.enter_context(tc.tile_pool(name="w0", bufs=1))
            wk_pool = s01.enter_context(tc.tile_pool(name="wk", bufs=1))
            wv_pool = s01.enter_context(tc.tile_pool(name="wv", bufs=1))
            nc.sync.dma_start(consts_sb[:], consts)
            wk_sb = wk_pool.tile([P, KO, H], F16, tag="wk")
            wkp = part6(wkt)
            nc.sync.dma_start(wk_sb[:, :, 0:P], wkp[:, :, 0:P])
            t = xt_pool.tile([P, KO, CH], F16, tag="xt0", name="xt0")
            nc.sync.dma_start(t[:], xtp[:, :, 0:CH])
            xtc[0] = t
            nc.sync.dma_start(wk_sb[:, :, P:H], wkp[:, :, P:H])
            xtq_sb = w0_pool.tile([P, KO, Q], F16, tag="xtq")
            nc.sync.dma_start(xtq_sb[:], part6(xtq))
            wq_sb = w0_pool.tile([P, KO, H], F16, tag="wq")
            nc.sync.dma_start(wq_sb[:], part6(wqt))
            wv_sb = wv_pool.tile([P, KO, H], F16, tag="wv")
            nc.sync.dma_start(wv_sb[:], part6(wvt))
            for c in range(1, NCH):
                t = xt_pool.tile([P, KO, CH], F16, tag=f"xt{c}", name=f"xt{c}")
                nc.sync.dma_start(t[:], xtp[:, :, c * CH:(c + 1) * CH])
                xtc[c] = t
            nc.sync.dma_start(lnr_sb[:], lnr)
            afit_sb = aw_pool.tile([P, KO, AD], F16, tag="afit")
            nc.sync.dma_start(afit_sb[:], part6(afit))
            aset_sb = aw_pool.tile([AD, H], F16, tag="aset")
            nc.sync.dma_start(aset_sb[:], aset)
            wi_sb = wi_pool.tile([P, KO, FF], F16, tag="wi")
            nc.sync.dma_start(wi_sb[:], part6(wit))

            kt_pool = s01.enter_context(tc.tile_pool(name="kt", bufs=1))
            vp_pool = s01.enter_context(tc.tile_pool(name="vp", bufs=1))
            acc_pool = s01.enter_context(tc.tile_pool(name="acc", bufs=1))
            et_pool = s01.enter_context(tc.tile_pool(name="et", bufs=8))
            nrm_pool = s01.enter_context(tc.tile_pool(name="nrm", bufs=2))
            # PSUM budget (8 banks): sc pair-group tile 4 + kv scratch 2
            # + ctx accumulator 2
            kvp = s01.enter_context(tc.tile_pool(name="kvp", bufs=2, space="PSUM"))
            scp = s01.enter_context(tc.tile_pool(name="scp", bufs=2, space="PSUM"))
            cxp = s01.enter_context(tc.tile_pool(name="cxp", bufs=2, space="PSUM"))

            accs = [
                acc_pool.tile([VH, Q], F16, tag=f"acc{h}", name=f"acc{h}")
                for h in range(NH)
            ]

            def k_chain(c, ko, kt):
                kp = kvp.tile([P, CH], F32, tag="kv", name="kp")
                for k in range(KO):
                    nc.tensor.matmul(
                        kp[:],
                        wk_sb[:, k, ko * P:(ko + 1) * P],
                        xtc[c][:, k, :],
                        start=(k == 0),
                        stop=(k == KO - 1),
                    )
                nc.vector.tensor_scalar_add(
                    kt[:, ko, :], kp[:], bk_sb[:, ko:ko + 1]
                )

            def v_chain(c, j, half, vp):
                emc = em_sb[:, c * JT + j:c * JT + j + 1]
                if half == 0:
                    nc.vector.tensor_scalar_mul(
                        vp[:, j, :, AD], vp[:, j, :, AD], emc
                    )
                vq = kvp.tile([P, 6 * AD], F32, tag="kv", name="vq")
                for k in range(KO):
                    nc.tensor.matmul(
                        vq[:],
                        xtc[c][:, k, j * P:(j + 1) * P],
                        wv_sb[:, k, half * 6 * AD:(half + 1) * 6 * AD],
                        start=(k == 0),
                        stop=(k == KO - 1),
                    )
                nc.vector.tensor_scalar_mul(
                    vp[:, j, half * 6:(half + 1) * 6, 0:AD],
                    vq[:, 0:6 * AD].rearrange("p (h d) -> p h d", d=AD),
                    emc,
                )

            def new_kt(c):
                kt = kt_pool.tile(
                    [P, KO, CH], F16, tag=f"kt{c}", name=f"kt{c}"
                )
                return kt

            def new_vp(c):
                vp = vp_pool.tile(
                    [P, JT, NH, VH], F16, tag=f"vp{c}", name=f"vp{c}"
                )
                nc.vector.memset(vp[:, :, :, AD], 1.0)
                return vp

            # chunk 0 projections up front. Later chunks' projection
            # chains are interleaved between head pairs as PE filler:
            # chunk c+1's K+V chains run during chunk c's pairs, except
            # the last chunk, whose V chains run one loop earlier and
            # whose K chains run just-in-time inside its own pair loop
            # (pair t2 only needs K block t2) so even the last loop has
            # independent PE work to hide the exp round-trips.
            kts = {0: new_kt(0)}
            vps = {0: new_vp(0)}
            for ko in range(KO):
                k_chain(0, ko, kts[0])
            # Q projection (its inputs land after wk/xt0; ko-outer
            # chains through the shared 2-bank scratch pool)
            for ko in range(KO):
                qp = kvp.tile([P, Q], F32, tag="kv", name="qp")
                for k in range(KO):
                    nc.tensor.matmul(
                        qp[:],
                        wq_sb[:, k, ko * P:(ko + 1) * P],
                        xtq_sb[:, k, :],
                        start=(k == 0),
                        stop=(k == KO - 1),
                    )
                nc.scalar.activation(
                    QT[:, ko, :], qp[:], AF.Identity,
                    bias=bq_sb[:, ko:ko + 1],
                )
            for j in range(JT):
                for half in range(2):
                    v_chain(0, j, half, vps[0])
            CL = NCH - 1

            for c in range(NCH):
                first = c == 0
                last = c == CL
                kt = kts[c]
                vp = vps[c]
                if c + 1 < CL:
                    kts[c + 1] = new_kt(c + 1)
                    vps[c + 1] = new_vp(c + 1)
                    # 6 K chains + 8 V chains spread over 6 head pairs
                    filler = [("k", c + 1, ko) for ko in range(KO)] + [
                        ("v", c + 1, (j, half))
                        for j in range(JT)
                        for half in range(2)
                    ]
                elif c + 1 == CL:
                    # next chunk is the last: its half-0 V chains + first K
                    # chain here; its K chains and half-1 V chains run
                    # just-in-time inside its own pair loop below
                    kts[CL] = new_kt(CL)
                    vps[CL] = new_vp(CL)
                    filler = [("v", CL, (j, 0)) for j in range(JT)]
                    filler.append(("k", CL, 0))
                else:
                    # last chunk: K blocks are emitted just-in-time at the
                    # top of each pair body below
                    filler = []

                v3jit = {0: [0, 1], 1: [2], 2: [3]}
                for t2 in range(NH // 2):
                    if last and t2 + 1 < KO:
                        k_chain(CL, t2 + 1, kt)
                    if last:
                        for j in v3jit.get(t2, []):
                            v_chain(CL, j, 1, vp)
                    fo = t2
                    ets = {}
                    for g in range(2):       # 2 j-groups x 2 heads
                        for h01 in range(2):
                            po = h01 * DH
                            sc = scp.tile([P, 2, CH], F32, tag="sc")
                            for jj in range(2):
                                j = g * 2 + jj
                                nc.tensor.matmul(
                                    sc[:, jj, :],
                                    kt[po:po + DH, fo, j * P:(j + 1) * P],
                                    QT[po:po + DH, fo, :],
                                    start=True,
                                    stop=True,
                                )
                            et = et_pool.tile([P, 2, CH], F16, tag="et")
                            nc.scalar.activation(
                                et[:].rearrange("p a n -> p (a n)"),
                                sc[:].rearrange("p a n -> p (a n)"),
                                AF.Exp,
                                bias=gexp_col[:],
                                scale=0.125,
                            )
                            ets[(h01, g)] = et
                    for h01 in range(2):
                        h = 2 * t2 + h01
                        po = h01 * DH
                        cx = cxp.tile([VH, Q], F32, tag="cx")
                        for j in range(JT):
                            nc.tensor.matmul(
                                cx[:],
                                vp[:, j, h, :],
                                ets[(h01, j // 2)][:, j % 2, :],
                                start=(j == 0),
                                stop=(j == JT - 1),
                            )
                        if first:
                            nc.vector.tensor_copy(accs[h][:], cx[:])
                        else:
                            nc.vector.tensor_add(accs[h][:], accs[h][:], cx[:])
                        if last:
                            # normalize: aot = acc * (1/denom) + bv
                            rc = nrm_pool.tile([1, Q], F16, tag="rc")
                            nc.vector.reciprocal(rc[:], accs[h][AD:VH, :])
                            bc = cxp.tile([P, Q], F32, tag="cx", name="bc")
                            nc.tensor.matmul(
                                bc[:], ones_row[:], rc[:], start=True, stop=True
                            )
                            nc.vector.tensor_mul(
                                aot[fo][po:po + DH, :],
                                accs[h][0:AD, :],
                                bc[0:DH, :],
                            )
                            nc.vector.tensor_scalar_add(
                                aot[fo][po:po + DH, :],
                                aot[fo][po:po + DH, :],
                                bv_sb[po:po + DH, fo:fo + 1],
                            )
                    # interleave projection chains of a later chunk
                    n = len(filler)
                    lo = (t2 * n) // 6
                    hi = ((t2 + 1) * n) // 6
                    for kind, fc, arg in filler[lo:hi]:
                        if kind == "k":
                            k_chain(fc, arg, kts[fc])
                        else:
                            v_chain(fc, arg[0], arg[1], vps[fc])

        # ========== stage 2 + 3: adapter + FFN + LayerNorm ==========
        with ExitStack() as s23:
            small = s23.enter_context(tc.tile_pool(name="small", bufs=1))
            big23 = s23.enter_context(tc.tile_pool(name="big23", bufs=1))
            mid_pool = s23.enter_context(tc.tile_pool(name="mid", bufs=4))
            wo_pool = s23.enter_context(tc.tile_pool(name="wo", bufs=1))
            it_pool = s23.enter_context(tc.tile_pool(name="it", bufs=1))
            s231 = s23.enter_context(ExitStack())
            ps = s231.enter_context(tc.tile_pool(name="ps", bufs=2, space="PSUM"))

            wo_sb = wo_pool.tile([P, FFT, H], F16, tag="wo")
            nc.sync.dma_start(wo_sb[:], wot.rearrange("(kk p) n -> p kk n", p=P))

            # adapter down-projection + gelu -> aT [AD, Q]
            ap_ps = ps.tile([AD, Q], F32, tag="ps512")
            for k in range(KO):
                nc.tensor.matmul(
                    ap_ps[:],
                    afit_sb[:, k, :],
                    aot[k][:],
                    start=(k == 0),
                    stop=(k == KO - 1),
                )
            aT = mid_pool.tile([AD, Q], F16, tag="aT")
            nc.scalar.activation(aT[:], ap_ps[:], AF.Gelu, bias=afib_sb[:])

            # adapter up-projection + residual -> attn2T [H, Q]
            a2t = [
                big23.tile([P, Q], F16, tag=f"a2t{m}", name=f"a2t{m}")
                for m in range(KO)
            ]
            for m in range(KO):
                pp = ps.tile([P, Q], F32, tag="ps512")
                nc.tensor.matmul(
                    pp[:],
                    aset_sb[:, m * P:(m + 1) * P],
                    aT[:],
                    start=True,
                    stop=True,
                )
                nc.vector.scalar_tensor_tensor(
                    a2t[m][:], pp[:], aseb_sb[:, m:m + 1], aot[m][:],
                    ALU.add, ALU.add,
                )

            # FFN1: all 24 gelu(inter) tiles, kept resident for FFN2
            its = []
            for k in range(FFT):
                ip = ps.tile([P, Q], F32, tag="ps512")
                for k6 in range(KO):
                    nc.tensor.matmul(
                        ip[:],
                        wi_sb[:, k6, k * P:(k + 1) * P],
                        a2t[k6][:],
                        start=(k6 == 0),
                        stop=(k6 == KO - 1),
                    )
                it = it_pool.tile([P, Q], F16, tag=f"it{k}", name=f"it{k}")
                nc.scalar.activation(it[:], ip[:], AF.Gelu, bias=bi_sb[:, k:k + 1])
                its.append(it)

            # FFN2 m-outer + fused LayerNorm statistics (the adapter/FFN1
            # PSUM pool is closed here so abp can hold 2 blocks in flight)
            s231.close()
            yp_pool = s23.enter_context(tc.tile_pool(name="yp", bufs=2, space="PSUM"))
            lnp = s23.enter_context(tc.tile_pool(name="lnp", bufs=1, space="PSUM"))
            abp = s23.enter_context(tc.tile_pool(name="abp", bufs=4, space="PSUM"))
            yts = [
                big23.tile([P, Q], F16, tag=f"yt{m}", name=f"yt{m}")
                for m in range(KO)
            ]
            mu_ps = lnp.tile([1, Q], F32, tag="mu")
            sq_ps = lnp.tile([1, Q], F32, tag="sq")
            for m in range(KO):
                yp = yp_pool.tile([P, Q], F32, tag="yp")
                for k in range(FFT):
                    nc.tensor.matmul(
                        yp[:],
                        wo_sb[:, k, m * P:(m + 1) * P],
                        its[k][:],
                        start=(k == 0),
                        stop=(k == FFT - 1),
                    )
                nc.vector.scalar_tensor_tensor(
                    yts[m][:], yp[:], bo_sb[:, m:m + 1], a2t[m][:],
                    ALU.add, ALU.add,
                )
                sqt = mid_pool.tile([P, Q], F16, tag="sqt")
                nc.vector.tensor_mul(sqt[:], yts[m][:], yts[m][:])
                nc.tensor.matmul(
                    mu_ps[:], ones_col[:], yts[m][:],
                    start=(m == 0), stop=(m == KO - 1),
                )
                nc.tensor.matmul(
                    sq_ps[:], ones_col[:], sqt[:],
                    start=(m == 0), stop=(m == KO - 1),
                )

            # LayerNorm scalars: rs = 1/sqrt(var+eps), nm = -mu*rs
            mu = small.tile([1, Q], F32, tag="mu_sb")
            nc.vector.tensor_scalar_mul(mu[:], mu_ps[:], 1.0 / H)
            mu2 = small.tile([1, Q], F32, tag="mu2")
            nc.vector.tensor_mul(mu2[:], mu[:], mu[:])
            ms = small.tile([1, Q], F32, tag="ms_sb")
            nc.vector.scalar_tensor_tensor(
                ms[:], sq_ps[:], 1.0 / H, mu2[:], ALU.mult, ALU.subtract
            )  # variance = sq/H - mu^2
            sd = small.tile([1, Q], F32, tag="sd")
            nc.scalar.activation(sd[:], ms[:], AF.Sqrt, bias=eps_sb[:])
            rs = small.tile([1, Q], F16, tag="rs")
            nc.vector.reciprocal(rs[:], sd[:])
            nmo = small.tile([2, Q], F16, tag="nmo")
            nc.vector.memset(nmo[:], 1.0)
            nc.vector.scalar_tensor_tensor(
                nmo[0:1, :], mu[:], -1.0, rs[:], ALU.mult, ALU.mult
            )  # nm = -mu * rs

            # A = g (x) rs ; M = g (x) nm + b (x) 1 ; out = y*A + M
            outt_p = part6(outt)
            for m in range(KO):
                A = abp.tile([P, Q], F32, tag="ab")
                nc.tensor.matmul(
                    A[:], lnr_sb[0:1, m * P:(m + 1) * P], rs[:],
                    start=True, stop=True,
                )
                M = abp.tile([P, Q], F32, tag="ab")
                nc.tensor.matmul(
                    M[:], lnr_sb[:, m * P:(m + 1) * P], nmo[:],
                    start=True, stop=True,
                )
                t1 = mid_pool.tile([P, Q], F16, tag="t1")
                nc.vector.tensor_mul(t1[:], yts[m][:], A[:])
                ot = mid_pool.tile([P, Q], F16, tag="ot")
                nc.vector.tensor_add(ot[:], t1[:], M[:])
                nc.sync.dma_start(outt_p[:, m, :], ot[:])

    nc.compile()
    return nc


_NC_CACHE = None


def _get_nc():
    global _NC_CACHE
    if _NC_CACHE is None:
        _NC_CACHE = _build_nc()
    return _NC_CACHE


def make_in_maps(
    hidden_states, attention_mask, wq, bq, wk, bk, wv, bv,
    a_fi_w, a_fi_b, a_se_w, a_se_b, wi, bi, wo, bo, ln_g, ln_b,
):
    f = np.float32
    h16 = np.float16
    ca = np.ascontiguousarray

    def part_bias(v, n):  # [n*128] -> [128, n]
        return ca(np.asarray(v, f).reshape(n, P).T)

    shared = {
        "wqt": ca(np.asarray(wq, h16).T),
        "wkt": ca(np.asarray(wk, h16).T),
        "wvt": ca(np.asarray(wv, h16).T),
        "afit": ca(np.asarray(a_fi_w, h16).T),
        "aset": ca(np.asarray(a_se_w, h16).T),
        "wit": ca(np.asarray(wi, h16).T),
        "wot": ca(np.asarray(wo, h16).T),
        "lnr": ca(np.stack([np.asarray(ln_g, h16), np.asarray(ln_b, h16)])),
    }

    def _consts(mask_b):
        c = np.zeros((P, 83), f)
        c[:, 0:16] = np.exp(np.minimum(mask_b, 0.0)).reshape(S // P, P).T
        c[:, 16:22] = part_bias(bq, KO)
        c[:, 22:28] = part_bias(bk, KO)
        c[:, 28:34] = part_bias(bv, KO)
        c[:, 34:40] = part_bias(a_se_b, KO)
        c[:, 40:46] = part_bias(bo, KO)
        c[:, 58:82] = part_bias(bi, FFT)
        c[0:AD, 82] = np.asarray(a_fi_b, f)
        return c

    hs = np.asarray(hidden_states, f)
    am = np.asarray(attention_mask, f)
    in_maps = []
    for c in range(NCORES):
        b = c // (NCORES // B)
        q0 = (c % (NCORES // B)) * Q
        m = dict(shared)
        m["xt"] = ca(hs[b].T.astype(h16))
        m["xtq"] = ca(hs[b, q0:q0 + Q].T.astype(h16))
        m["consts"] = _consts(am[b, 0, 0])
        in_maps.append(m)
    return in_maps


def gather_out(results):
    out = np.empty((B, S, H), np.float32)
    for c in range(NCORES):
        b = c // (NCORES // B)
        q0 = (c % (NCORES // B)) * Q
        out[b, q0:q0 + Q, :] = results[c]["outt"].T
    return out


def kernel(**inputs):
    nc = _get_nc()
    in_maps = make_in_maps(**inputs)
    res = run_bass_kernel_spmd(nc, in_maps, core_ids=list(range(NCORES)))
    return gather_out(res.results)


# revision 54
# speedup vs baseline: 1.0327x; 1.0327x over previous
"""BertLayer (attention + adapter + FFN + LayerNorm) Trainium2 Bass kernel.

Sharding: 8 cores, pure SPMD (no collectives). Core c handles batch b=c//4
and query rows [q0, q0+512) with q0=(c%4)*512. Each core computes K/V for
its full batch locally (replicated within the 4-core batch group), then
attention / adapter / FFN / LayerNorm for its 512 rows.

All on-chip compute is done in the "transposed" orientation (feature dim
on partitions, token dim on the free axis) so that every matmul has its
contraction dim on partitions and no on-device transposes are needed.

Structure (v3):
- The attention mask is folded into V: V rows (and the denominator ones
  column appended to V) are scaled by exp(mask[kpos]), which is exact
  (softmax(s+m) = diag(e^m)exp(s)/sum(e^m exp(s))). The softmax exp then
  has a CONSTANT bias (-2 fp16-overflow guard, cancels in normalization),
  letting one ScalarE activation cover a [128, 4*512] PSUM tile (a head
  pair x 2 kpos tiles) - few, wide ACT ops on the critical engine.
- Attention is chunk-major: for each 512-kpos chunk, all 12 heads'
  scores+exp+context run, with per-head context (and softmax denominator)
  accumulated across chunks into SBUF fp16 tiles by VectorE. K/V
  projection chains for chunk c+1 are interleaved between the head pairs
  of chunk c, so the PE always has independent work while ScalarE runs
  the exp chain.
- Scores matmuls for head pairs (2h, 2h+1) use disjoint 64-row groups of
  the PE array (base partitions 0/64) and separate PSUM banks, so the
  hardware runs them concurrently.
- All matmul operands are fp16 with fp32 PSUM accumulation. (fp8 was
  tried and rejected: the pre-LayerNorm signal is small, so LayerNorm
  renormalizes and fp8's ~5% per-projection error passes through to the
  output at full relative strength - measured 7e-2 vs the 2e-2 gate.)
- LayerNorm is folded into two small matmuls: A = g (x) rs and
  M = g (x) nm + b (x) 1 (rank-1/2 broadcasts), leaving 2 VectorE
  tensor-tensor ops per output block. FFN2 is m-outer (all gelu(it_k)
  tiles kept resident) so the residual/square/mean-accumulation work
  pipelines during FFN2 instead of forming a serial tail.
"""

import numpy as np

import concourse.bass as bass
import concourse.mybir as mybir
import concourse.tile as tile
from concourse import bacc
from concourse.bass_utils import run_bass_kernel_spmd
from contextlib import ExitStack

F32 = mybir.dt.float32
F16 = mybir.dt.float16
AF = mybir.ActivationFunctionType
ALU = mybir.AluOpType

B, S, H = 2, 2048, 768
NH, DH = 12, 64
FF = 3072
AD = 64
EPS = 1e-12
P = 128
KO = H // P          # 6 partition-tiles of the hidden dim
Q = 512              # query rows per core
NCORES = 8
NCH = 4              # kpos chunks (512 each)
CH = S // NCH        # 512
JT = CH // P         # 4 kpos 128-tiles per chunk
FFT = FF // P        # 24
VH = 65              # per-head V columns incl. exp(mask) column
GEXP = -2.0          # exp overflow guard; cancels in normalization


def _build_nc():
    nc = bacc.Bacc(
        "TRN2",
        target_bir_lowering=False,
        debug=False,
        num_devices=NCORES,
    )

    def din(name, shape, dt=F32):
        return nc.dram_tensor(name, list(shape), dt, kind="ExternalInput").ap()

    xt = din("xt", (H, S), F16)         # hidden[b].T
    xtq = din("xtq", (H, Q), F16)       # hidden[b, q0:q0+Q].T
    wqt = din("wqt", (H, H), F16)
    wkt = din("wkt", (H, H), F16)
    wvt = din("wvt", (H, H), F16)
    afit = din("afit", (H, AD), F16)
    aset = din("aset", (AD, H), F16)
    wit = din("wit", (H, FF), F16)
    wot = din("wot", (FF, H), F16)
    consts = din("consts", (P, 83))
    lnr = din("lnr", (2, H), F16)      # row0 = ln_g, row1 = ln_b
    outt = nc.dram_tensor("outt", [H, Q], F16, kind="ExternalOutput").ap()

    def part6(ap):  # [(ko p), n] -> [p, ko, n]
        return ap.rearrange("(ko p) n -> p ko n", p=P)

    with tile.TileContext(nc) as tc, nc.allow_low_precision(
        reason="fp8/fp16 matmul operands; accumulation stays fp32 in PSUM"
    ), ExitStack() as top:
        const = top.enter_context(tc.tile_pool(name="const", bufs=1))
        persist = top.enter_context(tc.tile_pool(name="persist", bufs=1))

        consts_sb = const.tile([P, 83], F32, tag="consts")
        em_sb = consts_sb[:, 0:16]         # exp(mask) per kpos tile
        bq_sb = consts_sb[:, 16:22]
        bk_sb = consts_sb[:, 22:28]
        bv_sb = consts_sb[:, 28:34]
        aseb_sb = consts_sb[:, 34:40]
        bo_sb = consts_sb[:, 40:46]
        bi_sb = consts_sb[:, 58:82]
        afib_sb = consts_sb[0:AD, 82:83]
        ones_col = const.tile([P, 1], F16, tag="ones")
        nc.vector.memset(ones_col[:], 1.0)
        ones_row = const.tile([1, P], F16, tag="ones_row")
        nc.vector.memset(ones_row[:], 1.0)
        gexp_col = const.tile([P, 1], F32, tag="gexp")
        nc.vector.memset(gexp_col[:], GEXP)
        eps_sb = const.tile([1, 1], F32, tag="eps")
        nc.vector.memset(eps_sb[:], EPS)
        lnr_sb = const.tile([2, H], F16, tag="lnr")
        nmo = const.tile([2, Q], F16, tag="nmo")
        nc.vector.memset(nmo[:], 1.0)

        # normalized attention output, transposed [H, Q]
        aot = [
            persist.tile([P, Q], F16, tag=f"aot{m}", name=f"aot{m}")
            for m in range(KO)
        ]
        aw_pool = top.enter_context(tc.tile_pool(name="aw", bufs=1))
        wi_pool = top.enter_context(tc.tile_pool(name="wi", bufs=1))

        # ========== stage 0 + 1: QKV projections + attention ==========
        with ExitStack() as s01:
            xt_pool = s01.enter_context(tc.tile_pool(name="xt", bufs=1))
            qt_pool = s01.enter_context(tc.tile_pool(name="qt", bufs=1))
            QT = qt_pool.tile([P, KO, Q], F16, tag="QT")

            xtp = part6(xt)
            xtc = {}

            # --- DMAs, ordered so K-projection (first PE work) unblocks
            # earliest: wk + the first xt chunk, then Q/V inputs
            w0_pool = s01.enter_context(tc.tile_pool(name="w0", bufs=1))
            wk_pool = s01.enter_context(tc.tile_pool(name="wk", bufs=1))
            wv_pool = s01.enter_context(tc.tile_pool(name="wv", bufs=1))
            nc.sync.dma_start(consts_sb[:], consts)
            wk_sb = wk_pool.tile([P, KO, H], F16, tag="wk")
            wkp = part6(wkt)
            nc.sync.dma_start(wk_sb[:, :, 0:P], wkp[:, :, 0:P])
            t = xt_pool.tile([P, KO, CH], F16, tag="xt0", name="xt0")
            nc.sync.dma_start(t[:], xtp[:, :, 0:CH])
            xtc[0] = t
            nc.sync.dma_start(wk_sb[:, :, P:H], wkp[:, :, P:H])
            xtq_sb = w0_pool.tile([P, KO, Q], F16, tag="xtq")
            nc.sync.dma_start(xtq_sb[:], part6(xtq))
            wq_sb = w0_pool.tile([P, KO, H], F16, tag="wq")
            nc.sync.dma_start(wq_sb[:], part6(wqt))
            wv_sb = wv_pool.tile([P, KO, H], F16, tag="wv")
            nc.sync.dma_start(wv_sb[:], part6(wvt))
            for c in range(1, NCH):
                t = xt_pool.tile([P, KO, CH], F16, tag=f"xt{c}", name=f"xt{c}")
                nc.sync.dma_start(t[:], xtp[:, :, c * CH:(c + 1) * CH])
                xtc[c] = t
            nc.sync.dma_start(lnr_sb[:], lnr)
            afit_sb = aw_pool.tile([P, KO, AD], F16, tag="afit")
            nc.sync.dma_start(afit_sb[:], part6(afit))
            aset_sb = aw_pool.tile([AD, H], F16, tag="aset")
            nc.sync.dma_start(aset_sb[:], aset)
            wi_sb = wi_pool.tile([P, KO, FF], F16, tag="wi")
            nc.sync.dma_start(wi_sb[:], part6(wit))

            kt_pool = s01.enter_context(tc.tile_pool(name="kt", bufs=1))
            vp_pool = s01.enter_context(tc.tile_pool(name="vp", bufs=1))
            acc_pool = s01.enter_context(tc.tile_pool(name="acc", bufs=1))
            et_pool = s01.enter_context(tc.tile_pool(name="et", bufs=8))
            nrm_pool = s01.enter_context(tc.tile_pool(name="nrm", bufs=2))
            # PSUM budget (8 banks): sc pair-group tile 4 + kv scratch 2
            # + ctx accumulator 2
            kvp = s01.enter_context(tc.tile_pool(name="kvp", bufs=2, space="PSUM"))
            scp = s01.enter_context(tc.tile_pool(name="scp", bufs=2, space="PSUM"))
            cxp = s01.enter_context(tc.tile_pool(name="cxp", bufs=2, space="PSUM"))

            accs = [
                acc_pool.tile([VH, Q], F16, tag=f"acc{h}", name=f"acc{h}")
                for h in range(NH)
            ]

            def k_chain(c, ko, kt):
                kp = kvp.tile([P, CH], F32, tag="kv", name="kp")
                for k in range(KO):
                    nc.tensor.matmul(
                        kp[:],
                        wk_sb[:, k, ko * P:(ko + 1) * P],
                        xtc[c][:, k, :],
                        start=(k == 0),
                        stop=(k == KO - 1),
                    )
                nc.vector.tensor_scalar_add(
                    kt[:, ko, :], kp[:], bk_sb[:, ko:ko + 1]
                )

            def v_chain(c, j, half, vp):
                emc = em_sb[:, c * JT + j:c * JT + j + 1]
                if half == 0:
                    nc.vector.tensor_scalar_mul(
                        vp[:, j, :, AD], vp[:, j, :, AD], emc
                    )
                vq = kvp.tile([P, 6 * AD], F32, tag="kv", name="vq")
                for k in range(KO):
                    nc.tensor.matmul(
                        vq[:],
                        xtc[c][:, k, j * P:(j + 1) * P],
                        wv_sb[:, k, half * 6 * AD:(half + 1) * 6 * AD],
                        start=(k == 0),
                        stop=(k == KO - 1),
                    )
                nc.vector.tensor_scalar_mul(
                    vp[:, j, half * 6:(half + 1) * 6, 0:AD],
                    vq[:, 0:6 * AD].rearrange("p (h d) -> p h d", d=AD),
                    emc,
                )

            def new_kt(c):
                kt = kt_pool.tile(
                    [P, KO, CH], F16, tag=f"kt{c}", name=f"kt{c}"
                )
                return kt

            def new_vp(c):
                vp = vp_pool.tile(
                    [P, JT, NH, VH], F16, tag=f"vp{c}", name=f"vp{c}"
                )
                nc.vector.memset(vp[:, :, :, AD], 1.0)
                return vp

            # chunk 0 projections up front. Later chunks' projection
            # chains are interleaved between head pairs as PE filler:
            # chunk c+1's K+V chains run during chunk c's pairs, except
            # the last chunk, whose V chains run one loop earlier and
            # whose K chains run just-in-time inside its own pair loop
            # (pair t2 only needs K block t2) so even the last loop has
            # independent PE work to hide the exp round-trips.
            kts = {0: new_kt(0)}
            vps = {0: new_vp(0)}
            for ko in range(KO):
                k_chain(0, ko, kts[0])
            # Q projection (its inputs land after wk/xt0; ko-outer
            # chains through the shared 2-bank scratch pool)
            for ko in range(KO):
                qp = kvp.tile([P, Q], F32, tag="kv", name="qp")
                for k in range(KO):
                    nc.tensor.matmul(
                        qp[:],
                        wq_sb[:, k, ko * P:(ko + 1) * P],
                        xtq_sb[:, k, :],
                        start=(k == 0),
                        stop=(k == KO - 1),
                    )
                nc.scalar.activation(
                    QT[:, ko, :], qp[:], AF.Identity,
                    bias=bq_sb[:, ko:ko + 1],
                )
            for j in range(JT):
                for half in range(2):
                    v_chain(0, j, half, vps[0])
            CL = NCH - 1

            for c in range(NCH):
                first = c == 0
                last = c == CL
                kt = kts[c]
                vp = vps[c]
                if c + 1 < CL:
                    kts[c + 1] = new_kt(c + 1)
                    vps[c + 1] = new_vp(c + 1)
                    # 6 K chains + 8 V chains spread over 6 head pairs
                    filler = [("k", c + 1, ko) for ko in range(KO)] + [
                        ("v", c + 1, (j, half))
                        for j in range(JT)
                        for half in range(2)
                    ]
                elif c + 1 == CL:
                    # next chunk is the last: its half-0 V chains + first K
                    # chain here; its K chains and half-1 V chains run
                    # just-in-time inside its own pair loop below
                    kts[CL] = new_kt(CL)
                    vps[CL] = new_vp(CL)
                    filler = [("v", CL, (j, 0)) for j in range(JT)]
                    filler.append(("k", CL, 0))
                else:
                    # last chunk: K blocks are emitted just-in-time at the
                    # top of each pair body below
                    filler = []

                v3jit = {0: [0, 1], 1: [2], 2: [3]}
                for t2 in range(NH // 2):
                    if last and t2 + 1 < KO:
                        k_chain(CL, t2 + 1, kt)
                    if last:
                        for j in v3jit.get(t2, []):
                            v_chain(CL, j, 1, vp)
                    fo = t2
                    ets = {}
                    for g in range(2):       # 2 j-groups x 2 heads
                        for h01 in range(2):
                            po = h01 * DH
                            sc = scp.tile([P, 2, CH], F32, tag="sc")
                            for jj in range(2):
                                j = g * 2 + jj
                                nc.tensor.matmul(
                                    sc[:, jj, :],
                                    kt[po:po + DH, fo, j * P:(j + 1) * P],
                                    QT[po:po + DH, fo, :],
                                    start=True,
                                    stop=True,
                                )
                            et = et_pool.tile([P, 2, CH], F16, tag="et")
                            nc.scalar.activation(
                                et[:].rearrange("p a n -> p (a n)"),
                                sc[:].rearrange("p a n -> p (a n)"),
                                AF.Exp,
                                bias=gexp_col[:],
                                scale=0.125,
                            )
                            ets[(h01, g)] = et
                    for h01 in range(2):
                        h = 2 * t2 + h01
                        po = h01 * DH
                        cx = cxp.tile([VH, Q], F32, tag="cx")
                        for j in range(JT):
                            nc.tensor.matmul(
                                cx[:],
                                vp[:, j, h, :],
                                ets[(h01, j // 2)][:, j % 2, :],
                                start=(j == 0),
                                stop=(j == JT - 1),
                            )
                        if first:
                            nc.vector.tensor_copy(accs[h][:], cx[:])
                        else:
                            nc.vector.tensor_add(accs[h][:], accs[h][:], cx[:])
                        if last:
                            # normalize: aot = acc * (1/denom) + bv
                            rc = nrm_pool.tile([1, Q], F16, tag="rc")
                            nc.vector.reciprocal(rc[:], accs[h][AD:VH, :])
                            bc = cxp.tile([P, Q], F32, tag="cx", name="bc")
                            nc.tensor.matmul(
                                bc[:], ones_row[:], rc[:], start=True, stop=True
                            )
                            nc.vector.tensor_mul(
                                aot[fo][po:po + DH, :],
                                accs[h][0:AD, :],
                                bc[0:DH, :],
                            )
                            nc.vector.tensor_scalar_add(
                                aot[fo][po:po + DH, :],
                                aot[fo][po:po + DH, :],
                                bv_sb[po:po + DH, fo:fo + 1],
                            )
                    # interleave projection chains of a later chunk
                    n = len(filler)
                    lo = (t2 * n) // 6
                    hi = ((t2 + 1) * n) // 6
                    for kind, fc, arg in filler[lo:hi]:
                        if kind == "k":
                            k_chain(fc, arg, kts[fc])
                        else:
                            v_chain(fc, arg[0], arg[1], vps[fc])

        # ========== stage 2 + 3: adapter + FFN + LayerNorm ==========
        with ExitStack() as s23:
            small = s23.enter_context(tc.tile_pool(name="small", bufs=1))
            big23 = s23.enter_context(tc.tile_pool(name="big23", bufs=1))
            mid_pool = s23.enter_context(tc.tile_pool(name="mid", bufs=4))
            wo_pool = s23.enter_context(tc.tile_pool(name="wo", bufs=1))
            it_pool = s23.enter_context(tc.tile_pool(name="it", bufs=1))
            s231 = s23.enter_context(ExitStack())
            ps = s231.enter_context(tc.tile_pool(name="ps", bufs=2, space="PSUM"))

            wo_sb = wo_pool.tile([P, FFT, H], F16, tag="wo")
            nc.sync.dma_start(wo_sb[:], wot.rearrange("(kk p) n -> p kk n", p=P))

            # adapter down-projection + gelu -> aT [AD, Q]
            ap_ps = ps.tile([AD, Q], F32, tag="ps512")
            for k in range(KO):
                nc.tensor.matmul(
                    ap_ps[:],
                    afit_sb[:, k, :],
                    aot[k][:],
                    start=(k == 0),
                    stop=(k == KO - 1),
                )
            aT = mid_pool.tile([AD, Q], F16, tag="aT")
            nc.scalar.activation(aT[:], ap_ps[:], AF.Gelu, bias=afib_sb[:])

            # adapter up-projection + residual -> attn2T [H, Q]
            a2t = [
                big23.tile([P, Q], F16, tag=f"a2t{m}", name=f"a2t{m}")
                for m in range(KO)
            ]
            for m in range(KO):
                pp = ps.tile([P, Q], F32, tag="ps512")
                nc.tensor.matmul(
                    pp[:],
                    aset_sb[:, m * P:(m + 1) * P],
                    aT[:],
                    start=True,
                    stop=True,
                )
                nc.vector.scalar_tensor_tensor(
                    a2t[m][:], pp[:], aseb_sb[:, m:m + 1], aot[m][:],
                    ALU.add, ALU.add,
                )

            # FFN1: all 24 gelu(inter) tiles, kept resident for FFN2
            its = []
            for k in range(FFT):
                ip = ps.tile([P, Q], F32, tag="ps512")
                for k6 in range(KO):
                    nc.tensor.matmul(
                        ip[:],
                        wi_sb[:, k6, k * P:(k + 1) * P],
                        a2t[k6][:],
                        start=(k6 == 0),
                        stop=(k6 == KO - 1),
                    )
                it = it_pool.tile([P, Q], F16, tag=f"it{k}", name=f"it{k}")
                nc.scalar.activation(it[:], ip[:], AF.Gelu, bias=bi_sb[:, k:k + 1])
                its.append(it)

            # FFN2 m-outer + fused LayerNorm statistics (the adapter/FFN1
            # PSUM pool is closed here so abp can hold 2 blocks in flight)
            s231.close()
            yp_pool = s23.enter_context(tc.tile_pool(name="yp", bufs=2, space="PSUM"))
            lnp = s23.enter_context(tc.tile_pool(name="lnp", bufs=1, space="PSUM"))
            abp = s23.enter_context(tc.tile_pool(name="abp", bufs=4, space="PSUM"))
            yts = [
                big23.tile([P, Q], F16, tag=f"yt{m}", name=f"yt{m}")
                for m in range(KO)
            ]
            mu_ps = lnp.tile([1, Q], F32, tag="mu")
            sq_ps = lnp.tile([1, Q], F32, tag="sq")
            for m in range(KO):
                yp = yp_pool.tile([P, Q], F32, tag="yp")
                for k in range(FFT):
                    nc.tensor.matmul(
                        yp[:],
                        wo_sb[:, k, m * P:(m + 1) * P],
                        its[k][:],
                        start=(k == 0),
                        stop=(k == FFT - 1),
                    )
                nc.vector.scalar_tensor_tensor(
                    yts[m][:], yp[:], bo_sb[:, m:m + 1], a2t[m][:],
                    ALU.add, ALU.add,
                )
                sqt = mid_pool.tile([P, Q], F16, tag="sqt")
                nc.vector.tensor_mul(sqt[:], yts[m][:], yts[m][:])
                nc.tensor.matmul(
                    mu_ps[:], ones_col[:], yts[m][:],
                    start=(m == 0), stop=(m == KO - 1),
                )
                nc.tensor.matmul(
                    sq_ps[:], ones_col[:], sqt[:],
                    start=(m == 0), stop=(m == KO - 1),
                )

            # LayerNorm scalars: rs = 1/sqrt(var+eps), nm = -mu*rs
            mu = small.tile([1, Q], F32, tag="mu_sb")
            nc.vector.tensor_scalar_mul(mu[:], mu_ps[:], 1.0 / H)
            mu2 = small.tile([1, Q], F32, tag="mu2")
            nc.vector.tensor_mul(mu2[:], mu[:], mu[:])
            ms = small.tile([1, Q], F32, tag="ms_sb")
            nc.vector.scalar_tensor_tensor(
                ms[:], sq_ps[:], 1.0 / H, mu2[:], ALU.mult, ALU.subtract
            )  # variance = sq/H - mu^2
            sd = small.tile([1, Q], F32, tag="sd")
            nc.scalar.activation(sd[:], ms[:], AF.Sqrt, bias=eps_sb[:])
            rs = small.tile([1, Q], F16, tag="rs")
            nc.vector.reciprocal(rs[:], sd[:])
            nc.vector.scalar_tensor_tensor(
                nmo[0:1, :], mu[:], -1.0, rs[:], ALU.mult, ALU.mult
            )  # nm = -mu * rs

            # A = g (x) rs ; M = g (x) nm + b (x) 1 ; out = y*A + M
            outt_p = part6(outt)
            for m in range(KO):
                A = abp.tile([P, Q], F32, tag="ab")
                nc.tensor.matmul(
                    A[:], lnr_sb[0:1, m * P:(m + 1) * P], rs[:],
                    start=True, stop=True,
                )
                M = abp.tile([P, Q], F32, tag="ab")
                nc.tensor.matmul(
                    M[:], lnr_sb[:, m * P:(m + 1) * P], nmo[:],
                    start=True, stop=True,
                )
                t1 = mid_pool.tile([P, Q], F16, tag="t1")
                nc.vector.tensor_mul(t1[:], yts[m][:], A[:])
                ot = mid_pool.tile([P, Q], F16, tag="ot")
                nc.vector.tensor_add(ot[:], t1[:], M[:])
                nc.sync.dma_start(outt_p[:, m, :], ot[:])

    nc.compile()
    return nc


_NC_CACHE = None


def _get_nc():
    global _NC_CACHE
    if _NC_CACHE is None:
        _NC_CACHE = _build_nc()
    return _NC_CACHE


def make_in_maps(
    hidden_states, attention_mask, wq, bq, wk, bk, wv, bv,
    a_fi_w, a_fi_b, a_se_w, a_se_b, wi, bi, wo, bo, ln_g, ln_b,
):
    f = np.float32
    h16 = np.float16
    ca = np.ascontiguousarray

    def part_bias(v, n):  # [n*128] -> [128, n]
        return ca(np.asarray(v, f).reshape(n, P).T)

    shared = {
        "wqt": ca(np.asarray(wq, h16).T),
        "wkt": ca(np.asarray(wk, h16).T),
        "wvt": ca(np.asarray(wv, h16).T),
        "afit": ca(np.asarray(a_fi_w, h16).T),
        "aset": ca(np.asarray(a_se_w, h16).T),
        "wit": ca(np.asarray(wi, h16).T),
        "wot": ca(np.asarray(wo, h16).T),
        "lnr": ca(np.stack([np.asarray(ln_g, h16), np.asarray(ln_b, h16)])),
    }

    def _consts(mask_b):
        c = np.zeros((P, 83), f)
        c[:, 0:16] = np.exp(np.minimum(mask_b, 0.0)).reshape(S // P, P).T
        c[:, 16:22] = part_bias(bq, KO)
        c[:, 22:28] = part_bias(bk, KO)
        c[:, 28:34] = part_bias(bv, KO)
        c[:, 34:40] = part_bias(a_se_b, KO)
        c[:, 40:46] = part_bias(bo, KO)
        c[:, 58:82] = part_bias(bi, FFT)
        c[0:AD, 82] = np.asarray(a_fi_b, f)
        return c

    hs = np.asarray(hidden_states, f)
    am = np.asarray(attention_mask, f)
    in_maps = []
    for c in range(NCORES):
        b = c // (NCORES // B)
        q0 = (c % (NCORES // B)) * Q
        m = dict(shared)
        m["xt"] = ca(hs[b].T.astype(h16))
        m["xtq"] = ca(hs[b, q0:q0 + Q].T.astype(h16))
        m["consts"] = _consts(am[b, 0, 0])
        in_maps.append(m)
    return in_maps


def gather_out(results):
    out = np.empty((B, S, H), np.float32)
    for c in range(NCORES):
        b = c // (NCORES // B)
        q0 = (c % (NCORES // B)) * Q
        out[b, q0:q0 + Q, :] = results[c]["outt"].T
    return out


def kernel(**inputs):
    nc = _get_nc()
    in_maps = make_in_maps(**inputs)
    res = run_bass_kernel_spmd(nc, in_maps, core_ids=list(range(NCORES)))
    return gather_out(res.results)
